# revision 8
# baseline (speedup 1.0000x reference)
"""GAT layer (nn_GATLayer_44220983279640) — Trainium2 Bass/Tile kernel.

Reference math per graph (B=16, D=512, FIN=FOUT=128, H=8):
    h  = x @ W                                         [D, F]
    s1[hd,i] = h[i] . a1[hd]   s2b[hd,j] = h[j] . a2[hd] + ab[hd]
    e  = leaky_relu(s1[:,None] + s2b[None,:])          [H, D, D]
    att = softmax_j(where(adj > 0, e, -9e15))
    out = mean_hd(att @ h)                             [D, F]

Sharding: data-parallel over batch, 2 graphs per core on 8 cores.

Key reformulation (exact math, no per-element exp):
  softmax rows may be rescaled arbitrarily. With per-row scale
  exp(-(s1_i + 2)) and exp(lrelu(x)) = max(exp(x), exp(0.01 x)):
      E'[j,i] = adj[j,i] * max(B_j, C_i * D_j)
      B_j = exp(s2b_j - 2)       (per-partition scalar slot, f32)
      C_i = exp(-0.99 s1_i - 2)  (row-broadcast fp16 tensor via DMA)
      D_j = exp(0.01 s2b_j)      (per-partition scalar slot, f32)
  The 4.2M-element exp/leaky_relu passes collapse into vector exps on
  [8,512] tensors in setup. Per head-graph the device does only:
    * 4x tensor_scalar   (C_bcast * D_j) max B_j  -> T1  (3 DVE + 1 GPSIMD)
    * tensor_tensor      T1 * adjmask             -> E'  (DVE 3/4, GPSIMD 1/4)
    * 16 PE matmuls      E'^T-slices @ [h/8 | 1]  -> U, rowsum (4 psum banks)
    * 1 DVE reciprocal   over the 4 rowsum columns (cross-bank strided AP)
    * 4x ACT Copy(U * 1/rowsum)  psum->sbuf fp16  -> U_norm
    * 1 PE matmul        I @ U_norm accumulating over heads in PSUM
  Graphs run back-to-back (b-outer): graph 1 setup and graph 0 output
  drain hide under the other graph's head loop. All C-broadcasts are
  prefetched with one big stride-0 DMA per graph.
"""

from contextlib import ExitStack

import numpy as np

import concourse.bass as bass
import concourse.bacc as bacc
import concourse.tile as tile
from concourse import mybir
from concourse.bass_utils import run_bass_kernel_spmd

B, D, FIN, FOUT, H = 16, 512, 128, 128, 8
NCORES = 8
NB = B // NCORES          # graphs per core
P = 128                   # partitions
NCH = D // P              # 4 j-chunks / i-tiles
DELTA = -2.0              # global exp downshift (cancels in softmax)
GPC = NCH - 1             # chunk index handled by GPSIMD

F32 = mybir.dt.float32
F16 = mybir.dt.float16

# packed f32 consts (columns): W | Wa1 | Wa2 | ab_row | ones_row | delta_col
CW0, CW1 = 0, FOUT
CA1 = CW1 + H                           # Wa1 = W @ a1^T  [FIN, H]
CA2 = CA1 + H                           # Wa2 = W @ a2^T  [FIN, H]
CAB = CA2 + H                           # ab row (partition 0)  [1, H]
CDL = CAB + P                           # delta column [P, 1] = DELTA
CONST_COLS = CDL + 1

_NC_CACHE = {}


def _build_bass():
    nc = bacc.Bacc("TRN2", debug=False, num_devices=NCORES)

    xT = nc.dram_tensor("xT", [NB, FIN, D], F32, kind="ExternalInput").ap()
    adjm = nc.dram_tensor("adjm", [NB, P, NCH * D], F16, kind="ExternalInput").ap()
    consts = nc.dram_tensor("consts", [P, CONST_COLS], F32, kind="ExternalInput").ap()
    ident = nc.dram_tensor("ident", [P, P], F16, kind="ExternalInput").ap()
    cd = nc.dram_tensor("cd", [NB, H, D], F16).ap()
    out = nc.dram_tensor("out", [NB, D, FOUT], F32, kind="ExternalOutput").ap()

    with tile.TileContext(nc) as tc, ExitStack() as ctx:
        _kernel_body(ctx, tc, out, xT, adjm, consts, ident, cd)
    nc.compile()
    return nc


def _kernel_body(ctx, tc, out, xT, adjm, consts, ident, cd):
    nc = tc.nc
    mult, vmax = mybir.AluOpType.mult, mybir.AluOpType.max
    Copy = mybir.ActivationFunctionType.Copy
    Exp = mybir.ActivationFunctionType.Exp

    const = ctx.enter_context(tc.tile_pool(name="const", bufs=1))
    xpool = ctx.enter_context(tc.tile_pool(name="xpool", bufs=NB))
    mpool = ctx.enter_context(tc.tile_pool(name="mpool", bufs=NB))
    hpool = ctx.enter_context(tc.tile_pool(name="hpool", bufs=NB))
    bdpool = ctx.enter_context(tc.tile_pool(name="bdpool", bufs=NB))
    crow = ctx.enter_context(tc.tile_pool(name="crow", bufs=NB))
    cbpool = ctx.enter_context(tc.tile_pool(name="cbpool", bufs=NB))
    t1pool = ctx.enter_context(tc.tile_pool(name="t1pool", bufs=3))
    epool = ctx.enter_context(tc.tile_pool(name="epool", bufs=3))
    unpool = ctx.enter_context(tc.tile_pool(name="unpool", bufs=3))
    rpool = ctx.enter_context(tc.tile_pool(name="rpool", bufs=4))
    aspool = ctx.enter_context(tc.tile_pool(name="aspool", bufs=NB))
    # PSUM: 2 setup + 4 agg + 2 head-accumulators = 8 banks
    pset = ctx.enter_context(tc.tile_pool(name="pset", bufs=2, space="PSUM"))
    pout = ctx.enter_context(tc.tile_pool(name="pout", bufs=4, space="PSUM"))
    pacc = ctx.enter_context(tc.tile_pool(name="pacc", bufs=NB, space="PSUM"))

    cst = const.tile([P, CONST_COLS], F32)
    nc.sync.dma_start(out=cst, in_=consts)
    I_sb = const.tile([P, P], F16)
    nc.sync.dma_start(out=I_sb, in_=ident)
    W_sb = cst[:, CW0:CW1]
    Wa1_sb = cst[:, CW1:CA1]
    Wa2_sb = cst[:, CA1:CA2]
    ab_row = cst[0:1, CA2:CA2 + H]
    ones_row = cst[0:1, CAB:CAB + P]
    dl_col = cst[:, CDL:CDL + 1]

    G = []  # per-graph state
    for b in range(NB):
        # --- per-graph setup (DMA issues spread: sync=x/cd/cb, scalar=m) ----
        x_sb = xpool.tile([FIN, D], F32, tag="x")
        nc.sync.dma_start(out=x_sb, in_=xT[b])
        m_sb = mpool.tile([P, NCH * D], F16, tag="m")
        nc.scalar.dma_start(out=m_sb, in_=adjm[b])

        # h tiles + ones column, fp16, h pre-scaled by 1/H
        haug = []
        for c in range(NCH):
            p_h = pset.tile([P, D], F32, tag="setup")
            nc.tensor.matmul(
                p_h[:, 0:FOUT], x_sb[:, bass.ts(c, P)], W_sb, start=True, stop=True
            )
            ha = hpool.tile([P, FOUT + 1], F16, tag=f"haug{c}")
            nc.scalar.activation(ha[:, 0:FOUT], p_h[:, 0:FOUT], Copy, scale=1.0 / H)
            nc.vector.memset(ha[:, FOUT:FOUT + 1], 1.0)
            haug.append(ha)

        # B/D per-partition scalars: s2bT[j, hd] = (x^T Wa2 + ab)[j, hd]
        B_col, D_col = [], []
        for c in range(NCH):
            p_s = pset.tile([P, D], F32, tag="setup")
            nc.tensor.matmul(
                p_s[:, 0:H], x_sb[:, bass.ts(c, P)], Wa2_sb, start=True, stop=False
            )
            nc.tensor.matmul(p_s[:, 0:H], ones_row, ab_row, start=False, stop=True)
            bc = bdpool.tile([P, H], F32, tag=f"B{c}")
            nc.scalar.activation(bc[:], p_s[:, 0:H], Exp, bias=dl_col)
            dc = bdpool.tile([P, H], F32, tag=f"D{c}")
            nc.scalar.activation(dc[:], p_s[:, 0:H], Exp, scale=0.01)
            B_col.append(bc)
            D_col.append(dc)

        # C row: exp(-0.99 * s1 + DELTA), staged to DRAM, then ALL heads'
        # broadcasts prefetched in two stride-0 DMAs (heads 0-1, heads 2-7)
        p_s1 = pset.tile([P, D], F32, tag="setup")
        nc.tensor.matmul(p_s1[0:H, :], Wa1_sb, x_sb[:], start=True, stop=True)
        c_sb = crow.tile([H, D], F16, tag="Crow")
        nc.scalar.activation(
            c_sb[:], p_s1[0:H, :], Exp, scale=-0.99, bias=dl_col[0:H, :]
        )
        nc.sync.dma_start(out=cd[b], in_=c_sb[:])

        cb_all = cbpool.tile([P, H, D], F16, tag="cb")
        row0 = cd[b, 0]
        for lo, hi in ((0, 2), (2, H)):
            nc.sync.dma_start(
                out=cb_all[:, lo:hi, :],
                in_=bass.AP(
                    tensor=cd.tensor, offset=row0.offset + lo * D,
                    ap=[[0, P], [D, hi - lo], row0.ap[-1]],
                ),
            )

        p_acc = pacc.tile([P, NCH * FOUT], F32, tag="acc")
        G.append(dict(
            m_sb=m_sb, haug=haug, B_col=B_col, D_col=D_col,
            cb_all=cb_all, p_acc=p_acc,
        ))

    # --- main head loops, one graph at a time --------------------------------
    for b in range(NB):
        g = G[b]
        m_sb, haug, cb_all = g["m_sb"], g["haug"], g["cb_all"]
        for hd in range(H):
            cb = cb_all[:, hd, :]
            # T1 = (C_i * D_j) max B_j ; chunk GPC on GPSIMD, rest on DVE
            t1 = t1pool.tile([P, GPC * D], F16, tag="t1")
            t1g = t1pool.tile([P, D], F16, tag="t1g")
            for c in range(NCH):
                eng = nc.gpsimd if c == GPC else nc.vector
                dst = t1g[:] if c == GPC else t1[:, bass.ts(c, D)]
                eng.tensor_scalar(
                    out=dst, in0=cb,
                    scalar1=g["D_col"][c][:, hd:hd + 1],
                    scalar2=g["B_col"][c][:, hd:hd + 1],
                    op0=mult, op1=vmax,
                )
            # E' = T1 * adjmask (DVE: chunks 0..2 in one op, GPSIMD: chunk 3)
            E = epool.tile([P, NCH * D], F16, tag="E")
            nc.vector.tensor_tensor(
                out=E[:, 0:GPC * D], in0=t1[:], in1=m_sb[:, 0:GPC * D], op=mult
            )
            nc.gpsimd.tensor_tensor(
                out=E[:, GPC * D:NCH * D], in0=t1g[:],
                in1=m_sb[:, GPC * D:NCH * D], op=mult,
            )

            # agg: psum[i-tile t] += E'^T-slice @ [h/8 | 1]
            p_os = []
            for t in range(NCH):
                p_o = pout.tile([P, FOUT + 1], F32, tag="po")
                for c in range(NCH):
                    nc.tensor.matmul(
                        p_o[:],
                        E[:, c * D + t * P: c * D + (t + 1) * P],
                        haug[c][:],
                        start=(c == 0),
                        stop=(c == NCH - 1),
                    )
                p_os.append(p_o)

            # one strided reciprocal across the 4 rowsum columns if the pout
            # banks are uniformly spaced, else 4 scalar reciprocals
            rall = rpool.tile([P, NCH], F32, tag="r")
            offs = [p_o[:, FOUT:FOUT + 1].offset for p_o in p_os]
            strides = {offs[t + 1] - offs[t] for t in range(NCH - 1)}
            if len(strides) == 1 and min(strides) > 0:
                src = bass.AP(
                    tensor=p_os[0].tensor, offset=offs[0],
                    ap=[list(p_os[0].ap[0]), [strides.pop(), NCH]],
                )
                nc.vector.reciprocal(rall[:], src)
            else:
                for t in range(NCH):
                    nc.vector.reciprocal(
                        rall[:, t:t + 1], p_os[t][:, FOUT:FOUT + 1]
                    )

            # normalize per head on ACT (scale = 1/rowsum), accumulate on PE
            un = unpool.tile([P, NCH * FOUT], F16, tag="un")
            for t in range(NCH):
                nc.scalar.activation(
                    un[:, bass.ts(t, FOUT)], p_os[t][:, 0:FOUT], Copy,
                    scale=rall[:, t:t + 1],
                )
            nc.tensor.matmul(
                g["p_acc"][:], I_sb, un[:], start=(hd == 0), stop=(hd == H - 1)
            )

        acc_sb = aspool.tile([P, NCH * FOUT], F32, tag="accsb")
        nc.scalar.activation(acc_sb[:], g["p_acc"][:], Copy)
        for t in range(NCH):
            nc.sync.dma_start(
                out=out[b, bass.ts(t, P), :], in_=acc_sb[:, bass.ts(t, FOUT)]
            )


def _prep_core_inputs(input, adj, W, a_w, a_b, core):
    gs = slice(core * NB, (core + 1) * NB)
    x_c = np.asarray(input[gs], dtype=np.float32)     # [NB, D, FIN]
    adj_c = np.asarray(adj[gs])                       # [NB, D, D] int32
    xT = np.ascontiguousarray(x_c.transpose(0, 2, 1))
    adjT = (adj_c.transpose(0, 2, 1) > 0)             # [NB, j, i]
    # [NB, j, i] -> [NB, p, c, i]  (j = c*128 + p)
    adjm = np.ascontiguousarray(
        adjT.reshape(NB, NCH, P, D).transpose(0, 2, 1, 3)
        .reshape(NB, P, NCH * D).astype(np.float16)
    )
    return {
        "xT": xT,
        "adjm": adjm,
        "consts": _pack_consts(W, a_w, a_b),
        "ident": np.eye(P, dtype=np.float16),
    }


def _pack_consts(W, a_w, a_b):
    W = np.asarray(W, dtype=np.float32)
    a_w = np.asarray(a_w, dtype=np.float32)
    a_b = np.asarray(a_b, dtype=np.float32)
    c = np.zeros((P, CONST_COLS), dtype=np.float32)
    c[:, CW0:CW1] = W
    c[:, CW1:CA1] = W @ a_w[:, :FOUT].T               # Wa1 [FIN, H]
    c[:, CA1:CA2] = W @ a_w[:, FOUT:].T               # Wa2 [FIN, H]
    c[0, CA2:CA2 + H] = a_b
    c[0, CAB:CAB + P] = 1.0
    c[:, CDL] = DELTA
    return c


def get_nc():
    if "nc" not in _NC_CACHE:
        _NC_CACHE["nc"] = _build_bass()
    return _NC_CACHE["nc"]


def run_on_device(in_maps, **kwargs):
    return run_bass_kernel_spmd(get_nc(), in_maps, list(range(NCORES)), **kwargs)


def kernel(input, adj, W, a_w, a_b):
    input = np.asarray(input, dtype=np.float32)
    adj = np.asarray(adj)

    in_maps = [
        _prep_core_inputs(input, adj, W, a_w, a_b, c) for c in range(NCORES)
    ]
    res = run_on_device(in_maps)
    outs = [res.results[c]["out"] for c in range(NCORES)]
    return np.concatenate(outs, axis=0).astype(np.float32)


if __name__ == "__main__":
    nc = get_nc()
    print("built ok")


# revision 9
# speedup vs baseline: 2.3262x; 2.3262x over previous
"""GAT layer (nn_GATLayer_44220983279640) — Trainium2 Bass/Tile kernel.

Reference math per graph (B=16, D=512, FIN=FOUT=128, H=8):
    h  = x @ W                                         [D, F]
    s1[hd,i] = h[i] . a1[hd]   s2b[hd,j] = h[j] . a2[hd] + ab[hd]
    e  = leaky_relu(s1[:,None] + s2b[None,:])          [H, D, D]
    att = softmax_j(where(adj > 0, e, -9e15))
    out = mean_hd(att @ h)                             [D, F]

Sharding: data-parallel over batch, 2 graphs per core on 8 cores.

Key reformulation (exact math, no per-element exp):
  softmax rows may be rescaled arbitrarily. With per-row scale
  exp(-(s1_i + 2)) and exp(lrelu(x)) = max(exp(x), exp(0.01 x)):
      E'[j,i] = adj[j,i] * max(B_j, C_i * D_j)
      B_j = exp(s2b_j - 2)       (per-partition scalar slot, f32)
      C_i = exp(-0.99 s1_i - 2)  (row-broadcast fp16 tensor via DMA)
      D_j = exp(0.01 s2b_j)      (per-partition scalar slot, f32)
  The 4.2M-element exp/leaky_relu passes collapse into vector exps on
  [8,512] tensors in setup. Per head-graph the device does only:
    * 4x tensor_scalar   (C_bcast * D_j) max B_j  -> T1  (3 DVE + 1 GPSIMD)
    * tensor_tensor      T1 * adjmask             -> E'  (DVE 3/4, GPSIMD 1/4)
    * 16 PE matmuls      E'^T-slices @ [h/8 | 1]  -> U, rowsum (4 psum banks)
    * 1 DVE reciprocal   over the 4 rowsum columns (cross-bank strided AP)
    * 4x ACT Copy(U * 1/rowsum)  psum->sbuf fp16  -> U_norm
    * 1 PE matmul        I @ U_norm accumulating over heads in PSUM
  Graphs run back-to-back (b-outer): graph 1 setup and graph 0 output
  drain hide under the other graph's head loop. All C-broadcasts are
  prefetched with one big stride-0 DMA per graph.
"""

from contextlib import ExitStack

import numpy as np

import concourse.bass as bass
import concourse.bacc as bacc
import concourse.tile as tile
from concourse import mybir
from concourse.bass_utils import run_bass_kernel_spmd

B, D, FIN, FOUT, H = 16, 512, 128, 128, 8
NCORES = 8
NB = B // NCORES          # graphs per core
P = 128                   # partitions
NCH = D // P              # 4 j-chunks / i-tiles
DELTA = -2.0              # global exp downshift (cancels in softmax)
GPC = NCH - 1             # chunk index handled by GPSIMD

F32 = mybir.dt.float32
F16 = mybir.dt.float16

# packed f32 consts (columns): W | Wa1 | Wa2 | ab_row | ones_row | delta_col
CW0, CW1 = 0, FOUT
CA1 = CW1 + H                           # Wa1 = W @ a1^T  [FIN, H]
CA2 = CA1 + H                           # Wa2 = W @ a2^T  [FIN, H]
CAB = CA2 + H                           # ab row (partition 0)  [1, H]
CDL = CAB + P                           # delta column [P, 1] = DELTA
CONST_COLS = CDL + 1

_NC_CACHE = {}


def _build_bass():
    nc = bacc.Bacc("TRN2", debug=False, num_devices=NCORES)

    xT = nc.dram_tensor("xT", [NB, FIN, D], F32, kind="ExternalInput").ap()
    adjm = nc.dram_tensor("adjm", [NB, P, NCH * D], F16, kind="ExternalInput").ap()
    consts = nc.dram_tensor("consts", [P, CONST_COLS], F32, kind="ExternalInput").ap()
    ident = nc.dram_tensor("ident", [P, P], F16, kind="ExternalInput").ap()
    cd = nc.dram_tensor("cd", [NB, H, D], F16).ap()
    out = nc.dram_tensor("out", [NB, D, FOUT], F32, kind="ExternalOutput").ap()

    with tile.TileContext(nc) as tc, ExitStack() as ctx:
        _kernel_body(ctx, tc, out, xT, adjm, consts, ident, cd)
    nc.compile()
    return nc


def _kernel_body(ctx, tc, out, xT, adjm, consts, ident, cd):
    nc = tc.nc
    mult, vmax = mybir.AluOpType.mult, mybir.AluOpType.max
    Copy = mybir.ActivationFunctionType.Copy
    Exp = mybir.ActivationFunctionType.Exp

    const = ctx.enter_context(tc.tile_pool(name="const", bufs=1))
    xpool = ctx.enter_context(tc.tile_pool(name="xpool", bufs=NB))
    mpool = ctx.enter_context(tc.tile_pool(name="mpool", bufs=NB))
    hpool = ctx.enter_context(tc.tile_pool(name="hpool", bufs=NB))
    bdpool = ctx.enter_context(tc.tile_pool(name="bdpool", bufs=NB))
    crow = ctx.enter_context(tc.tile_pool(name="crow", bufs=NB))
    cbpool = ctx.enter_context(tc.tile_pool(name="cbpool", bufs=NB))
    t1pool = ctx.enter_context(tc.tile_pool(name="t1pool", bufs=3))
    epool = ctx.enter_context(tc.tile_pool(name="epool", bufs=3))
    unpool = ctx.enter_context(tc.tile_pool(name="unpool", bufs=3))
    rpool = ctx.enter_context(tc.tile_pool(name="rpool", bufs=4))
    aspool = ctx.enter_context(tc.tile_pool(name="aspool", bufs=NB))
    # PSUM: 2 setup + 4 agg + 2 head-accumulators = 8 banks
    pset = ctx.enter_context(tc.tile_pool(name="pset", bufs=2, space="PSUM"))
    pout = ctx.enter_context(tc.tile_pool(name="pout", bufs=4, space="PSUM"))
    pacc = ctx.enter_context(tc.tile_pool(name="pacc", bufs=NB, space="PSUM"))

    cst = const.tile([P, CONST_COLS], F32)
    nc.sync.dma_start(out=cst, in_=consts)
    I_sb = const.tile([P, P], F16)
    nc.sync.dma_start(out=I_sb, in_=ident)
    W_sb = cst[:, CW0:CW1]
    Wa1_sb = cst[:, CW1:CA1]
    Wa2_sb = cst[:, CA1:CA2]
    ab_row = cst[0:1, CA2:CA2 + H]
    ones_row = cst[0:1, CAB:CAB + P]
    dl_col = cst[:, CDL:CDL + 1]

    G = []  # per-graph state
    for b in range(NB):
        # --- per-graph setup (DMA issues spread: sync=x/cd/cb, scalar=m) ----
        x_sb = xpool.tile([FIN, D], F32, tag="x")
        nc.sync.dma_start(out=x_sb, in_=xT[b])
        m_sb = mpool.tile([P, NCH * D], F16, tag="m")
        nc.scalar.dma_start(out=m_sb, in_=adjm[b])

        # h tiles + ones column, fp16, h pre-scaled by 1/H
        haug = []
        for c in range(NCH):
            p_h = pset.tile([P, D], F32, tag="setup")
            nc.tensor.matmul(
                p_h[:, 0:FOUT], x_sb[:, bass.ts(c, P)], W_sb, start=True, stop=True
            )
            ha = hpool.tile([P, FOUT + 1], F16, tag=f"haug{c}")
            nc.scalar.activation(ha[:, 0:FOUT], p_h[:, 0:FOUT], Copy, scale=1.0 / H)
            nc.vector.memset(ha[:, FOUT:FOUT + 1], 1.0)
            haug.append(ha)

        # B/D per-partition scalars: s2bT[j, hd] = (x^T Wa2 + ab)[j, hd]
        B_col, D_col = [], []
        for c in range(NCH):
            p_s = pset.tile([P, D], F32, tag="setup")
            nc.tensor.matmul(
                p_s[:, 0:H], x_sb[:, bass.ts(c, P)], Wa2_sb, start=True, stop=False
            )
            nc.tensor.matmul(p_s[:, 0:H], ones_row, ab_row, start=False, stop=True)
            bc = bdpool.tile([P, H], F32, tag=f"B{c}")
            nc.scalar.activation(bc[:], p_s[:, 0:H], Exp, bias=dl_col)
            dc = bdpool.tile([P, H], F32, tag=f"D{c}")
            nc.scalar.activation(dc[:], p_s[:, 0:H], Exp, scale=0.01)
            B_col.append(bc)
            D_col.append(dc)

        # C row: exp(-0.99 * s1 + DELTA), staged to DRAM, then ALL heads'
        # broadcasts prefetched in two stride-0 DMAs (heads 0-1, heads 2-7)
        p_s1 = pset.tile([P, D], F32, tag="setup")
        nc.tensor.matmul(p_s1[0:H, :], Wa1_sb, x_sb[:], start=True, stop=True)
        c_sb = crow.tile([H, D], F16, tag="Crow")
        nc.scalar.activation(
            c_sb[:], p_s1[0:H, :], Exp, scale=-0.99, bias=dl_col[0:H, :]
        )
        nc.sync.dma_start(out=cd[b], in_=c_sb[:])

        cb_all = cbpool.tile([P, H, D], F16, tag="cb")
        row0 = cd[b, 0]
        for lo, hi in ((0, 2), (2, H)):
            nc.sync.dma_start(
                out=cb_all[:, lo:hi, :],
                in_=bass.AP(
                    tensor=cd.tensor, offset=row0.offset + lo * D,
                    ap=[[0, P], [D, hi - lo], row0.ap[-1]],
                ),
            )

        p_acc = pacc.tile([P, NCH * FOUT], F32, tag="acc")
        G.append(dict(
            m_sb=m_sb, haug=haug, B_col=B_col, D_col=D_col,
            cb_all=cb_all, p_acc=p_acc,
        ))

    # --- main head loops, one graph at a time --------------------------------
    for b in range(NB):
        g = G[b]
        m_sb, haug, cb_all = g["m_sb"], g["haug"], g["cb_all"]
        for hd in range(H):
            cb = cb_all[:, hd, :]
            # T1 = (C_i * D_j) max B_j ; chunk GPC on GPSIMD, rest on DVE
            t1 = t1pool.tile([P, NCH * D], F16, tag="t1")
            for c in range(NCH):
                nc.vector.tensor_scalar(
                    out=t1[:, bass.ts(c, D)], in0=cb,
                    scalar1=g["D_col"][c][:, hd:hd + 1],
                    scalar2=g["B_col"][c][:, hd:hd + 1],
                    op0=mult, op1=vmax,
                )
            # E' = T1 * adjmask (one 2x-mode tensor_tensor)
            E = epool.tile([P, NCH * D], F16, tag="E")
            nc.vector.tensor_tensor(out=E[:], in0=t1[:], in1=m_sb[:], op=mult)

            # agg: psum[i-tile t] += E'^T-slice @ [h/8 | 1]
            p_os = []
            for t in range(NCH):
                p_o = pout.tile([P, FOUT + 1], F32, tag="po")
                for c in range(NCH):
                    nc.tensor.matmul(
                        p_o[:],
                        E[:, c * D + t * P: c * D + (t + 1) * P],
                        haug[c][:],
                        start=(c == 0),
                        stop=(c == NCH - 1),
                    )
                p_os.append(p_o)

            # one strided reciprocal across the 4 rowsum columns if the pout
            # banks are uniformly spaced, else 4 scalar reciprocals
            rall = rpool.tile([P, NCH], F32, tag="r")
            offs = [p_o[:, FOUT:FOUT + 1].offset for p_o in p_os]
            strides = {offs[t + 1] - offs[t] for t in range(NCH - 1)}
            if hd == 0 and b == 0:
                print("pout rowsum offsets:", offs, "strides:", strides)
            if len(strides) == 1 and min(strides) > 0:
                src = bass.AP(
                    tensor=p_os[0].tensor, offset=offs[0],
                    ap=[list(p_os[0].ap[0]), [strides.pop(), NCH]],
                )
                nc.vector.reciprocal(rall[:], src)
            else:
                for t in range(NCH):
                    nc.vector.reciprocal(
                        rall[:, t:t + 1], p_os[t][:, FOUT:FOUT + 1]
                    )

            # normalize per head on ACT (scale = 1/rowsum), accumulate on PE
            un = unpool.tile([P, NCH * FOUT], F16, tag="un")
            for t in range(NCH):
                nc.scalar.activation(
                    un[:, bass.ts(t, FOUT)], p_os[t][:, 0:FOUT], Copy,
                    scale=rall[:, t:t + 1],
                )
            nc.tensor.matmul(
                g["p_acc"][:], I_sb, un[:], start=(hd == 0), stop=(hd == H - 1)
            )

        acc_sb = aspool.tile([P, NCH * FOUT], F32, tag="accsb")
        nc.scalar.activation(acc_sb[:], g["p_acc"][:], Copy)
        for t in range(NCH):
            nc.sync.dma_start(
                out=out[b, bass.ts(t, P), :], in_=acc_sb[:, bass.ts(t, FOUT)]
            )


def _prep_core_inputs(input, adj, W, a_w, a_b, core):
    gs = slice(core * NB, (core + 1) * NB)
    x_c = np.asarray(input[gs], dtype=np.float32)     # [NB, D, FIN]
    adj_c = np.asarray(adj[gs])                       # [NB, D, D] int32
    xT = np.ascontiguousarray(x_c.transpose(0, 2, 1))
    adjT = (adj_c.transpose(0, 2, 1) > 0)             # [NB, j, i]
    # [NB, j, i] -> [NB, p, c, i]  (j = c*128 + p)
    adjm = np.ascontiguousarray(
        adjT.reshape(NB, NCH, P, D).transpose(0, 2, 1, 3)
        .reshape(NB, P, NCH * D).astype(np.float16)
    )
    return {
        "xT": xT,
        "adjm": adjm,
        "consts": _pack_consts(W, a_w, a_b),
        "ident": np.eye(P, dtype=np.float16),
    }


def _pack_consts(W, a_w, a_b):
    W = np.asarray(W, dtype=np.float32)
    a_w = np.asarray(a_w, dtype=np.float32)
    a_b = np.asarray(a_b, dtype=np.float32)
    c = np.zeros((P, CONST_COLS), dtype=np.float32)
    c[:, CW0:CW1] = W
    c[:, CW1:CA1] = W @ a_w[:, :FOUT].T               # Wa1 [FIN, H]
    c[:, CA1:CA2] = W @ a_w[:, FOUT:].T               # Wa2 [FIN, H]
    c[0, CA2:CA2 + H] = a_b
    c[0, CAB:CAB + P] = 1.0
    c[:, CDL] = DELTA
    return c


def get_nc():
    if "nc" not in _NC_CACHE:
        _NC_CACHE["nc"] = _build_bass()
    return _NC_CACHE["nc"]


def run_on_device(in_maps, **kwargs):
    return run_bass_kernel_spmd(get_nc(), in_maps, list(range(NCORES)), **kwargs)


def kernel(input, adj, W, a_w, a_b):
    input = np.asarray(input, dtype=np.float32)
    adj = np.asarray(adj)

    in_maps = [
        _prep_core_inputs(input, adj, W, a_w, a_b, c) for c in range(NCORES)
    ]
    res = run_on_device(in_maps)
    outs = [res.results[c]["out"] for c in range(NCORES)]
    return np.concatenate(outs, axis=0).astype(np.float32)


if __name__ == "__main__":
    nc = get_nc()
    print("built ok")


# revision 10
# speedup vs baseline: 2.5826x; 1.1102x over previous
"""GAT layer (nn_GATLayer_44220983279640) — Trainium2 Bass/Tile kernel.

Reference math per graph (B=16, D=512, FIN=FOUT=128, H=8):
    h  = x @ W                                         [D, F]
    s1[hd,i] = h[i] . a1[hd]   s2b[hd,j] = h[j] . a2[hd] + ab[hd]
    e  = leaky_relu(s1[:,None] + s2b[None,:])          [H, D, D]
    att = softmax_j(where(adj > 0, e, -9e15))
    out = mean_hd(att @ h)                             [D, F]

Sharding: data-parallel over batch, 2 graphs per core on 8 cores.

Key reformulation (exact math, no per-element exp):
  softmax rows may be rescaled arbitrarily. With per-row scale
  exp(-(s1_i + 2)) and exp(lrelu(x)) = max(exp(x), exp(0.01 x)):
      E'[j,i] = adj[j,i] * max(B_j, C_i * D_j)
      B_j = exp(s2b_j - 2)       (per-partition scalar slot, f32)
      C_i = exp(-0.99 s1_i - 2)  (row-broadcast fp16 tensor via DMA)
      D_j = exp(0.01 s2b_j)      (per-partition scalar slot, f32)
  The 4.2M-element exp/leaky_relu passes collapse into vector exps on
  [8,512] tensors in setup. Per head-graph the device does only:
    * 4x tensor_scalar   (C_bcast * D_j) max B_j  -> T1  (3 DVE + 1 GPSIMD)
    * tensor_tensor      T1 * adjmask             -> E'  (DVE 3/4, GPSIMD 1/4)
    * 16 PE matmuls      E'^T-slices @ [h/8 | 1]  -> U, rowsum (4 psum banks)
    * 1 DVE reciprocal   over the 4 rowsum columns (cross-bank strided AP)
    * 4x ACT Copy(U * 1/rowsum)  psum->sbuf fp16  -> U_norm
    * 1 PE matmul        I @ U_norm accumulating over heads in PSUM
  Graphs run back-to-back (b-outer): graph 1 setup and graph 0 output
  drain hide under the other graph's head loop. All C-broadcasts are
  prefetched with one big stride-0 DMA per graph.
"""

from contextlib import ExitStack

import numpy as np

import concourse.bass as bass
import concourse.bacc as bacc
import concourse.tile as tile
from concourse import mybir
from concourse.bass_utils import run_bass_kernel_spmd

B, D, FIN, FOUT, H = 16, 512, 128, 128, 8
NCORES = 8
NB = B // NCORES          # graphs per core
P = 128                   # partitions
NCH = D // P              # 4 j-chunks / i-tiles
DELTA = -2.0              # global exp downshift (cancels in softmax)
GPC = NCH - 1             # chunk index handled by GPSIMD

F32 = mybir.dt.float32
F16 = mybir.dt.float16

# packed f32 consts (columns): W | Wa1 | Wa2 | ab_row | ones_row | delta_col
CW0, CW1 = 0, FOUT
CA1 = CW1 + H                           # Wa1 = W @ a1^T  [FIN, H]
CA2 = CA1 + H                           # Wa2 = W @ a2^T  [FIN, H]
CAB = CA2 + H                           # ab row (partition 0)  [1, H]
CDL = CAB + P                           # delta column [P, 1] = DELTA
CONST_COLS = CDL + 1

_NC_CACHE = {}


def _build_bass():
    nc = bacc.Bacc("TRN2", debug=False, num_devices=NCORES)

    xT = nc.dram_tensor("xT", [NB, FIN, D], F16, kind="ExternalInput").ap()
    adjm = nc.dram_tensor("adjm", [NB, P, NCH * D], F16, kind="ExternalInput").ap()
    consts = nc.dram_tensor("consts", [P, CONST_COLS], F32, kind="ExternalInput").ap()
    constsH = nc.dram_tensor("constsH", [P, FOUT + 2 * H], F16, kind="ExternalInput").ap()
    ident = nc.dram_tensor("ident", [P, P], F16, kind="ExternalInput").ap()
    cd = nc.dram_tensor("cd", [NB, H, D], F16).ap()
    out = nc.dram_tensor("out", [NB, D, FOUT], F32, kind="ExternalOutput").ap()

    with tile.TileContext(nc) as tc, ExitStack() as ctx:
        _kernel_body(ctx, tc, out, xT, adjm, consts, constsH, ident, cd)
    nc.compile()
    return nc


def _kernel_body(ctx, tc, out, xT, adjm, consts, constsH, ident, cd):
    nc = tc.nc
    mult, vmax = mybir.AluOpType.mult, mybir.AluOpType.max
    Copy = mybir.ActivationFunctionType.Copy
    Exp = mybir.ActivationFunctionType.Exp

    const = ctx.enter_context(tc.tile_pool(name="const", bufs=1))
    xpool = ctx.enter_context(tc.tile_pool(name="xpool", bufs=NB))
    mpool = ctx.enter_context(tc.tile_pool(name="mpool", bufs=NB))
    hpool = ctx.enter_context(tc.tile_pool(name="hpool", bufs=NB))
    bdpool = ctx.enter_context(tc.tile_pool(name="bdpool", bufs=NB))
    crow = ctx.enter_context(tc.tile_pool(name="crow", bufs=NB))
    cbpool = ctx.enter_context(tc.tile_pool(name="cbpool", bufs=NB))
    t1pool = ctx.enter_context(tc.tile_pool(name="t1pool", bufs=3))
    epool = ctx.enter_context(tc.tile_pool(name="epool", bufs=3))
    unpool = ctx.enter_context(tc.tile_pool(name="unpool", bufs=3))
    rpool = ctx.enter_context(tc.tile_pool(name="rpool", bufs=4))
    aspool = ctx.enter_context(tc.tile_pool(name="aspool", bufs=NB))
    # PSUM: 2 setup + 4 agg + 2 head-accumulators = 8 banks
    pset = ctx.enter_context(tc.tile_pool(name="pset", bufs=2, space="PSUM"))
    pout = ctx.enter_context(tc.tile_pool(name="pout", bufs=4, space="PSUM"))
    pacc = ctx.enter_context(tc.tile_pool(name="pacc", bufs=NB, space="PSUM"))

    cst = const.tile([P, CONST_COLS], F32)
    nc.sync.dma_start(out=cst, in_=consts)
    csth = const.tile([P, FOUT + 2 * H], F16)
    nc.sync.dma_start(out=csth, in_=constsH)
    I_sb = const.tile([P, P], F16)
    nc.scalar.dma_start(out=I_sb, in_=ident)
    W_sb = csth[:, CW0:CW1]
    Wa1_sb = csth[:, CW1:CA1]
    Wa2_sb = csth[:, CA1:CA2]
    ab_row = cst[0:1, CA2:CA2 + H]
    ones_row = cst[0:1, CAB:CAB + P]
    dl_col = cst[:, CDL:CDL + 1]

    G = []  # per-graph state
    for b in range(NB):
        # --- per-graph setup (DMA issues spread: sync=x/cd/cb, scalar=m) ----
        x_sb = xpool.tile([FIN, D], F16, tag="x")
        nc.sync.dma_start(out=x_sb, in_=xT[b])
        m_sb = mpool.tile([P, NCH * D], F16, tag="m")
        nc.scalar.dma_start(out=m_sb, in_=adjm[b])

        # C chain first: it gates the first tensor_scalar of the head loop.
        # C row: exp(-0.99 * s1 + DELTA), staged to DRAM, then all heads'
        # broadcasts prefetched in two stride-0 DMAs (head 0, heads 1-7)
        p_s1 = pset.tile([P, D], F32, tag="setup")
        nc.tensor.matmul(p_s1[0:H, :], Wa1_sb, x_sb[:], start=True, stop=True)
        c_sb = crow.tile([H, D], F16, tag="Crow")
        nc.scalar.activation(
            c_sb[:], p_s1[0:H, :], Exp, scale=-0.99, bias=dl_col[0:H, :]
        )
        nc.sync.dma_start(out=cd[b], in_=c_sb[:])

        cb_all = cbpool.tile([P, H, D], F16, tag="cb")
        row0 = cd[b, 0]
        for lo, hi in ((0, 1), (1, H)):
            nc.sync.dma_start(
                out=cb_all[:, lo:hi, :],
                in_=bass.AP(
                    tensor=cd.tensor, offset=row0.offset + lo * D,
                    ap=[[0, P], [D, hi - lo], row0.ap[-1]],
                ),
            )

        # B/D per-partition scalars: s2bT[j, hd] = (x^T Wa2 + ab)[j, hd]
        B_col, D_col = [], []
        for c in range(NCH):
            p_s = pset.tile([P, D], F32, tag="setup")
            nc.tensor.matmul(
                p_s[:, 0:H], x_sb[:, bass.ts(c, P)], Wa2_sb, start=True, stop=False
            )
            nc.tensor.matmul(p_s[:, 0:H], ones_row, ab_row, start=False, stop=True)
            bc = bdpool.tile([P, H], F32, tag=f"B{c}")
            nc.scalar.activation(bc[:], p_s[:, 0:H], Exp, bias=dl_col)
            dc = bdpool.tile([P, H], F32, tag=f"D{c}")
            nc.scalar.activation(dc[:], p_s[:, 0:H], Exp, scale=0.01)
            B_col.append(bc)
            D_col.append(dc)

        # h tiles + ones column, fp16, h pre-scaled by 1/H
        haug = []
        for c in range(NCH):
            p_h = pset.tile([P, D], F32, tag="setup")
            nc.tensor.matmul(
                p_h[:, 0:FOUT], x_sb[:, bass.ts(c, P)], W_sb, start=True, stop=True
            )
            ha = hpool.tile([P, FOUT + 1], F16, tag=f"haug{c}")
            nc.scalar.activation(ha[:, 0:FOUT], p_h[:, 0:FOUT], Copy, scale=1.0 / H)
            nc.vector.memset(ha[:, FOUT:FOUT + 1], 1.0)
            haug.append(ha)

        p_acc = pacc.tile([P, NCH * FOUT], F32, tag="acc")
        G.append(dict(
            m_sb=m_sb, haug=haug, B_col=B_col, D_col=D_col,
            cb_all=cb_all, p_acc=p_acc,
        ))

    # --- main head loops, one graph at a time --------------------------------
    for b in range(NB):
        g = G[b]
        m_sb, haug, cb_all = g["m_sb"], g["haug"], g["cb_all"]
        for hd in range(H):
            cb = cb_all[:, hd, :]
            # T1 = (C_i * D_j) max B_j ; chunk GPC on GPSIMD, rest on DVE
            t1 = t1pool.tile([P, NCH * D], F16, tag="t1")
            for c in range(NCH):
                nc.vector.tensor_scalar(
                    out=t1[:, bass.ts(c, D)], in0=cb,
                    scalar1=g["D_col"][c][:, hd:hd + 1],
                    scalar2=g["B_col"][c][:, hd:hd + 1],
                    op0=mult, op1=vmax,
                )
            # E' = T1 * adjmask (one 2x-mode tensor_tensor)
            E = epool.tile([P, NCH * D], F16, tag="E")
            nc.vector.tensor_tensor(out=E[:], in0=t1[:], in1=m_sb[:], op=mult)

            # agg: psum[i-tile t] += E'^T-slice @ [h/8 | 1]
            p_os = []
            for t in range(NCH):
                p_o = pout.tile([P, FOUT + 1], F32, tag="po")
                for c in range(NCH):
                    nc.tensor.matmul(
                        p_o[:],
                        E[:, c * D + t * P: c * D + (t + 1) * P],
                        haug[c][:],
                        start=(c == 0),
                        stop=(c == NCH - 1),
                    )
                p_os.append(p_o)

            # one strided reciprocal across the 4 rowsum columns if the pout
            # banks are uniformly spaced, else 4 scalar reciprocals
            rall = rpool.tile([P, NCH], F32, tag="r")
            offs = [p_o[:, FOUT:FOUT + 1].offset for p_o in p_os]
            strides = {offs[t + 1] - offs[t] for t in range(NCH - 1)}
            if hd == 0 and b == 0:
                print("pout rowsum offsets:", offs, "strides:", strides)
            if len(strides) == 1 and min(strides) > 0:
                src = bass.AP(
                    tensor=p_os[0].tensor, offset=offs[0],
                    ap=[list(p_os[0].ap[0]), [strides.pop(), NCH]],
                )
                nc.vector.reciprocal(rall[:], src)
            else:
                for t in range(NCH):
                    nc.vector.reciprocal(
                        rall[:, t:t + 1], p_os[t][:, FOUT:FOUT + 1]
                    )

            # normalize per head on ACT (scale = 1/rowsum), accumulate on PE
            un = unpool.tile([P, NCH * FOUT], F16, tag="un")
            for t in range(NCH):
                nc.scalar.activation(
                    un[:, bass.ts(t, FOUT)], p_os[t][:, 0:FOUT], Copy,
                    scale=rall[:, t:t + 1],
                )
            nc.tensor.matmul(
                g["p_acc"][:], I_sb, un[:], start=(hd == 0), stop=(hd == H - 1)
            )

        acc_sb = aspool.tile([P, NCH, FOUT], F32, tag="accsb")
        nc.scalar.activation(acc_sb[:], g["p_acc"][:], Copy)
        ob = out[b]
        nc.sync.dma_start(
            out=bass.AP(
                tensor=out.tensor, offset=ob.offset,
                ap=[[FOUT, P], [P * FOUT, NCH], [1, FOUT]],
            ),
            in_=acc_sb[:],
        )


def _prep_core_inputs(input, adj, W, a_w, a_b, core):
    gs = slice(core * NB, (core + 1) * NB)
    x_c = np.asarray(input[gs], dtype=np.float32)     # [NB, D, FIN]
    adj_c = np.asarray(adj[gs])                       # [NB, D, D] int32
    xT = np.ascontiguousarray(x_c.transpose(0, 2, 1)).astype(np.float16)
    adjT = (adj_c.transpose(0, 2, 1) > 0)             # [NB, j, i]
    # [NB, j, i] -> [NB, p, c, i]  (j = c*128 + p)
    adjm = np.ascontiguousarray(
        adjT.reshape(NB, NCH, P, D).transpose(0, 2, 1, 3)
        .reshape(NB, P, NCH * D).astype(np.float16)
    )
    return {
        "xT": xT,
        "adjm": adjm,
        "consts": _pack_consts(W, a_w, a_b),
        "constsH": _pack_consts_h(W, a_w),
        "ident": np.eye(P, dtype=np.float16),
    }


def _pack_consts_h(W, a_w):
    W = np.asarray(W, dtype=np.float32)
    a_w = np.asarray(a_w, dtype=np.float32)
    c = np.zeros((P, FOUT + 2 * H), dtype=np.float32)
    c[:, CW0:CW1] = W
    c[:, CW1:CA1] = W @ a_w[:, :FOUT].T               # Wa1 [FIN, H]
    c[:, CA1:CA2] = W @ a_w[:, FOUT:].T               # Wa2 [FIN, H]
    return c.astype(np.float16)


def _pack_consts(W, a_w, a_b):
    W = np.asarray(W, dtype=np.float32)
    a_w = np.asarray(a_w, dtype=np.float32)
    a_b = np.asarray(a_b, dtype=np.float32)
    c = np.zeros((P, CONST_COLS), dtype=np.float32)
    c[:, CW0:CW1] = W
    c[:, CW1:CA1] = W @ a_w[:, :FOUT].T               # Wa1 [FIN, H]
    c[:, CA1:CA2] = W @ a_w[:, FOUT:].T               # Wa2 [FIN, H]
    c[0, CA2:CA2 + H] = a_b
    c[0, CAB:CAB + P] = 1.0
    c[:, CDL] = DELTA
    return c


def get_nc():
    if "nc" not in _NC_CACHE:
        _NC_CACHE["nc"] = _build_bass()
    return _NC_CACHE["nc"]


def run_on_device(in_maps, **kwargs):
    return run_bass_kernel_spmd(get_nc(), in_maps, list(range(NCORES)), **kwargs)


def kernel(input, adj, W, a_w, a_b):
    input = np.asarray(input, dtype=np.float32)
    adj = np.asarray(adj)

    in_maps = [
        _prep_core_inputs(input, adj, W, a_w, a_b, c) for c in range(NCORES)
    ]
    res = run_on_device(in_maps)
    outs = [res.results[c]["out"] for c in range(NCORES)]
    return np.concatenate(outs, axis=0).astype(np.float32)


if __name__ == "__main__":
    nc = get_nc()
    print("built ok")
